# revision 26
# baseline (speedup 1.0000x reference)
"""CQAttention (context-query attention) Trainium2 kernel.

Problem (per batch b of 16):
    S  = (C@w1)[:,None] + (Q@w2)[None,:] + (C*w3)@Q^T          [Lc, Lq]
    S1 = softmax_j(S masked by qmask), S2 = softmax_i(S masked by cmask)
    A  = S1@Q ;  Z = S2^T@C ;  Bm = S1@Z
    out = [C, A, C*A, C*Bm] @ out_w^T + out_b                  [Lc, d]
with B=16, Lc=1024, Lq=512, d=512, fp32.

Sharding: data-parallel over batch, 2 batches per NeuronCore, no
collectives. Each core runs the full per-batch pipeline.

Device mapping notes:
- Softmaxes skip max-subtraction (logits are O(1)); masking is folded in
  as additive -1e4 biases so exp underflows to exactly 0 at masked
  positions. E (=exp of masked logits) is produced in both layouts:
  natural [Lc,Lq] (cmask-masked, for Z and its column sums) and
  transposed [Lq,Lc] (qmask-masked, for A/Bm and row sums), each via its
  own PE pass over the same fp32r operands (C^T and w3*Q^T).
- The rank-1 logit terms ride the PE: c1=C@w1 / q2=Q@w2 are computed as
  [1,N] rows, then folded into each logit matmul group as one extra K=1
  accumulation matmul (partition-direction term) and one DVE add of a
  [128,N] broadcast tile (free-direction term). Broadcast tiles are
  built on-chip with K=1 replicate matmuls (ones_row^T @ row), so there
  are no DRAM round-trips anywhere on the critical path.
- Mask biases enter as per-partition ACT-bias columns prepared on host.
- Softmax normalizations: 1/colsum is multiplied into E_cm in place
  (making S2), 1/rowsum into E^T (making S1^T); both reciprocal rows are
  replicated on the PE.
- The final linear is computed transposed (out^T = OW^T-tiles @ out4^T)
  so every matmul operand is already in the right layout; the host
  transposes the [d, Lc] result back.
- All matmul operands are float32r (full PE rate at N>=256, ~11-bit
  mantissa); accumulation is fp32 in PSUM.
- split_multi_waits works around this container's walrus, which rejects
  any instruction carrying more than one sync wait.
"""

import numpy as np

import concourse.bass as bass
import concourse.mybir as mybir
import concourse.tile as tile
from concourse.bass_utils import run_bass_kernel_spmd

F32 = mybir.dt.float32
F32R = mybir.dt.float32r
AF = mybir.ActivationFunctionType

B, LC, LQ, D = 16, 1024, 512, 512
NCORES = 8
BPC = B // NCORES  # batches per core
I_T, J_T, K_T = LC // 128, LQ // 128, D // 128  # 8, 4, 4
F_T = 4 * D // 128  # 16 feature tiles of out4
MASK_BIAS = 1.0e4  # exp(x - 1e4) == 0.0 exactly in fp32 for |x| ~ O(10)

SECTIONS = []


def _mark(nc, label):
    SECTIONS.append((label, int(nc.get_next_instruction_name().split("-")[1])))


def split_multi_waits(nc):
    """This walrus build allows at most one sync wait per instruction;
    hoist extras onto standalone EventSemaphore (wait) instructions."""
    for f in nc.m.functions:
        for blk in f.blocks:
            new = []
            changed = False
            for inst in blk.instructions:
                si = inst.sync_info
                waits = list(si.on_wait) if si is not None else []
                if len(waits) > 1:
                    changed = True
                    for k, w in enumerate(waits[:-1]):
                        ev = mybir.InstEventSemaphore(
                            name=f"{inst.name}-sw{k}", ins=[], outs=[]
                        )
                        ev.engine = inst.engine
                        ev.sync_info = mybir.SyncInfo(on_wait=[w], on_update=[])
                        new.append(ev)
                    si.on_wait = [waits[-1]]
                    inst.sync_info = si
                new.append(inst)
            if changed:
                blk.instructions = new


def _emit_front(nc, tc, pools, consts, dram, b):
    (sb, small, psum, rowps) = pools
    (ones_r, ones_row, w1c, w2c, w3c, ow, obc) = consts

    _mark(nc, f"b{b}.inputs")
    # ---- input tiles (qt first: q2 unblocks PE earliest) ----
    qt = []
    for j in range(J_T):
        t = sb.tile([128, LQ], F32R, tag="qt", bufs=4, name=f"qt{j}")
        nc.sync.dma_start(out=t[:], in_=dram["q_t"].ap()[b, j * 128:(j + 1) * 128, :])
        qt.append(t)
    ct = []
    for k in range(K_T):
        t = sb.tile([128, LC], F32R, tag="ct", bufs=8, name=f"ct{k}")
        nc.sync.dma_start(out=t[:], in_=dram["c_t"].ap()[b, k * 128:(k + 1) * 128, :])
        ct.append(t)
    cb_col = small.tile([128, I_T], F32, tag="cb_col", bufs=2)
    nc.scalar.dma_start(out=cb_col[:], in_=dram["cb_col"].ap()[b])
    qb_col = small.tile([128, J_T], F32, tag="qb_col", bufs=2)
    nc.scalar.dma_start(out=qb_col[:], in_=dram["qb_col"].ap()[b])

    _mark(nc, f"b{b}.q2")
    # ---- q2 = Q@w2 row + its [128,LQ] broadcast ----
    q2_ps = rowps.tile([1, LQ], F32, tag="rowps", name="q2ps")
    for k in range(K_T):
        nc.tensor.matmul(q2_ps[:], w2c[:, k:k + 1], qt[k][:],
                         start=(k == 0), stop=(k == K_T - 1))
    q2_row = small.tile([1, LQ], F32R, tag="q2_row", bufs=2)
    nc.scalar.copy(q2_row[:], q2_ps[:])

    _mark(nc, f"b{b}.c1")
    # ---- c1 = C@w1 rows + [128,LC] broadcast ----
    c1_rows = []
    for n in range(2):
        c1_ps = rowps.tile([1, 512], F32, tag="rowps", name=f"c1ps{n}")
        for k in range(K_T):
            nc.tensor.matmul(c1_ps[:], w1c[:, k:k + 1],
                             ct[k][:, n * 512:(n + 1) * 512],
                             start=(k == 0), stop=(k == K_T - 1))
        c1_row = small.tile([1, 512], F32R, tag="c1_row", bufs=2, name=f"c1row{n}")
        nc.scalar.copy(c1_row[:], c1_ps[:])
        c1_rows.append(c1_row)

    _mark(nc, f"b{b}.qw3t")
    # ---- QW3^T = Q^T * w3 (per-partition scale) ----
    qw3t = []
    for k in range(K_T):
        t = sb.tile([128, LQ], F32R, tag="qw3t", bufs=4, name=f"qw3t{k}")
        nc.vector.tensor_scalar_mul(t[:], qt[k][:], w3c[:, k:k + 1])
        qw3t.append(t)

    return dict(qt=qt, ct=ct, cb_col=cb_col, qb_col=qb_col, q2_row=q2_row,
                c1_rows=c1_rows, qw3t=qw3t)


def _emit_back(nc, tc, pools, consts, dram, b, fr):
    (sb, small, psum, rowps) = pools
    (ones_r, ones_row, w1c, w2c, w3c, ow, obc) = consts
    qt, ct = fr["qt"], fr["ct"]
    cb_col, qb_col = fr["cb_col"], fr["qb_col"]
    q2_row, c1_rows, qw3t = fr["q2_row"], fr["c1_rows"], fr["qw3t"]

    _mark(nc, f"b{b}.ecm")
    # ---- E_cm (natural): exp(S + cmask bias), colsum, normalize -> S2 ----
    ecm = []
    cs_ps = rowps.tile([1, LQ], F32, tag="rowps", name="csps")
    for i in range(I_T):
        s_ps = psum.tile([128, LQ], F32, tag="mmps", name=f"sps{i}")
        for k in range(K_T):
            nc.tensor.matmul(s_ps[:], ct[k][:, i * 128:(i + 1) * 128], qw3t[k][:],
                             start=(k == 0), stop=False)
        c1r = c1_rows[i // 4]
        nc.tensor.matmul(s_ps[:], c1r[:1, (i % 4) * 128:(i % 4 + 1) * 128],
                         ones_row[:], start=False, stop=False)
        nc.tensor.matmul(s_ps[:], ones_row[:1, :128], q2_row[:],
                         start=False, stop=True)
        e = sb.tile([128, LQ], F32R, tag="ecm", bufs=8, name=f"ecm{i}")
        nc.scalar.activation(e[:], s_ps[:], AF.Exp,
                             bias=cb_col[:, i:i + 1], scale=1.0)
        ecm.append(e)
        nc.tensor.matmul(cs_ps[:], ones_r[:], e[:],
                         start=(i == 0), stop=(i == I_T - 1))
    cs_row = small.tile([1, LQ], F32R, tag="cs_row", bufs=2)
    nc.scalar.copy(cs_row[:], cs_ps[:])
    with nc.allow_low_precision(reason="f32r rounding of softmax denominators"):
        nc.vector.tensor_scalar_add(cs_row[:], cs_row[:], 1e-30)
        nc.vector.reciprocal(cs_row[:], cs_row[:])
    ics_ps = psum.tile([128, LQ], F32, tag="mmps", name="icsps")
    nc.tensor.matmul(ics_ps[:], ones_row[:1, :128], cs_row[:], start=True, stop=True)
    ics_bcast = sb.tile([128, LQ], F32, tag="ics_bcast", bufs=1)
    nc.scalar.copy(ics_bcast[:], ics_ps[:])
    for i in range(I_T):
        nc.vector.tensor_mul(ecm[i][:], ecm[i][:], ics_bcast[:])

    cn = []
    for i in range(I_T):
        t = sb.tile([128, D], F32R, tag="cn", bufs=8, name=f"cn{i}")
        nc.gpsimd.dma_start(out=t[:], in_=dram["c_nat"].ap()[b, i * 128:(i + 1) * 128, :])
        cn.append(t)

    _mark(nc, f"b{b}.et")
    # ---- E^T (transposed): exp(S^T + qmask bias) -> S1^T via 1/rowsum ----
    et = [sb.tile([128, LC], F32R, tag="et", bufs=4, name=f"et{_j}")
          for _j in range(J_T)]
    irs_bcast = sb.tile([128, LC], F32, tag="irs_bcast", bufs=1)
    for n in range(2):
        for j in range(J_T):
            st_ps = psum.tile([128, 512], F32, tag="mmps", name=f"stps{n}_{j}")
            for k in range(K_T):
                nc.tensor.matmul(st_ps[:], qw3t[k][:, j * 128:(j + 1) * 128],
                                 ct[k][:, n * 512:(n + 1) * 512],
                                 start=(k == 0), stop=False)
            nc.tensor.matmul(st_ps[:], q2_row[:1, j * 128:(j + 1) * 128],
                             ones_row[:], start=False, stop=False)
            nc.tensor.matmul(st_ps[:], ones_row[:1, :128], c1_rows[n][:],
                             start=False, stop=True)
            nc.scalar.activation(et[j][:, n * 512:(n + 1) * 512], st_ps[:], AF.Exp,
                                 bias=qb_col[:, j:j + 1], scale=1.0)
        rs_ps = rowps.tile([1, 512], F32, tag="rowps", name=f"rsps{n}")
        for j in range(J_T):
            nc.tensor.matmul(rs_ps[:], ones_r[:],
                             et[j][:, n * 512:(n + 1) * 512],
                             start=(j == 0), stop=(j == J_T - 1))
        rs_row = small.tile([1, 512], F32R, tag="rs_row", bufs=2, name=f"rsrow{n}")
        nc.scalar.copy(rs_row[:], rs_ps[:])
        with nc.allow_low_precision(reason="f32r rounding of softmax denominators"):
            nc.vector.reciprocal(rs_row[:], rs_row[:])
        irs_ps = psum.tile([128, 512], F32, tag="mmps", name=f"irsps{n}")
        nc.tensor.matmul(irs_ps[:], ones_row[:1, :128], rs_row[:],
                         start=True, stop=True)
        nc.scalar.copy(irs_bcast[:, n * 512:(n + 1) * 512], irs_ps[:])

    _mark(nc, f"b{b}.z")
    # ---- Z = S2^T @ C ----
    z = []
    for j in range(J_T):
        z_ps = psum.tile([128, D], F32, tag="mmps", name=f"zps{j}")
        for i in range(I_T):
            nc.tensor.matmul(z_ps[:], ecm[i][:, j * 128:(j + 1) * 128], cn[i][:],
                             start=(i == 0), stop=(i == I_T - 1))
        zt = sb.tile([128, D], F32R, tag="z", bufs=4, name=f"z{j}")
        nc.scalar.copy(zt[:], z_ps[:])
        z.append(zt)

    qn = []
    for j in range(J_T):
        t = sb.tile([128, D], F32R, tag="qn", bufs=4, name=f"qn{j}")
        nc.gpsimd.dma_start(out=t[:], in_=dram["q_nat"].ap()[b, j * 128:(j + 1) * 128, :])
        qn.append(t)

    _mark(nc, f"b{b}.s1t")
    # ---- S1^T = E^T / rowsum (in place, per chunk) ----
    for n in range(2):
        sl = slice(n * 512, (n + 1) * 512)
        for j in range(J_T):
            nc.vector.tensor_mul(et[j][:, sl], et[j][:, sl], irs_bcast[:, sl])

    _mark(nc, f"b{b}.ab")
    # ---- per n-chunk: A^T, Bm^T, C*A, C*Bm staging, then the out matmuls ----
    for n in range(2):
        sl = slice(n * 512, (n + 1) * 512)
        at_n, cat_n, cbt_n = [], [], []
        for m in range(K_T):
            a_ps = psum.tile([128, 512], F32, tag="mmps", name=f"aps{n}_{m}")
            for j in range(J_T):
                nc.tensor.matmul(a_ps[:], qn[j][:, m * 128:(m + 1) * 128],
                                 et[j][:, sl],
                                 start=(j == 0), stop=(j == J_T - 1))
            at = sb.tile([128, 512], F32R, tag="at", bufs=4, name=f"at{m}_{n}")
            nc.vector.tensor_copy(at[:], a_ps[:])
            at_n.append(at)
            b_ps = psum.tile([128, 512], F32, tag="mmps", name=f"bps{n}_{m}")
            for j in range(J_T):
                nc.tensor.matmul(b_ps[:], z[j][:, m * 128:(m + 1) * 128],
                                 et[j][:, sl],
                                 start=(j == 0), stop=(j == J_T - 1))
            cbt = sb.tile([128, 512], F32R, tag="cbt", bufs=4, name=f"cbt{m}_{n}")
            nc.vector.tensor_copy(cbt[:], b_ps[:])
            cbt_n.append(cbt)
            cat = sb.tile([128, 512], F32R, tag="cat", bufs=4, name=f"cat{m}_{n}")
            nc.vector.tensor_mul(cat[:], ct[m][:, sl], at[:])
            cat_n.append(cat)
            nc.vector.tensor_mul(cbt[:], ct[m][:, sl], cbt[:])

        _mark(nc, f"b{b}.out{n}")
        for m in range(K_T):
            o_ps = psum.tile([128, 512], F32, tag="mmps", name=f"ops{n}_{m}")
            for f in range(F_T):
                g, k = f // 4, f % 4
                if g == 0:
                    rhs = ct[k][:, sl]
                elif g == 1:
                    rhs = at_n[k][:]
                elif g == 2:
                    rhs = cat_n[k][:]
                else:
                    rhs = cbt_n[k][:]
                nc.tensor.matmul(o_ps[:], ow[f][:, m * 128:(m + 1) * 128], rhs,
                                 start=(f == 0), stop=(f == F_T - 1))
            ot = sb.tile([128, 512], F32, tag="ot", bufs=2, name=f"ot{m}_{n}")
            nc.scalar.activation(ot[:], o_ps[:], AF.Identity,
                                 bias=obc[:, m:m + 1], scale=1.0)
            nc.sync.dma_start(
                out=dram["out_t"].ap()[b, m * 128:(m + 1) * 128,
                                       n * 512:(n + 1) * 512],
                in_=ot[:])


def build():
    nc = bass.Bass("TRN2", target_bir_lowering=False, debug=False,
                   num_devices=NCORES)
    dram = {}
    dram["c_nat"] = nc.dram_tensor("c_nat", [BPC, LC, D], F32R, kind="ExternalInput")
    dram["c_t"] = nc.dram_tensor("c_t", [BPC, D, LC], F32R, kind="ExternalInput")
    dram["q_nat"] = nc.dram_tensor("q_nat", [BPC, LQ, D], F32R, kind="ExternalInput")
    dram["q_t"] = nc.dram_tensor("q_t", [BPC, D, LQ], F32R, kind="ExternalInput")
    dram["cb_col"] = nc.dram_tensor("cb_col", [BPC, 128, I_T], F32, kind="ExternalInput")
    dram["qb_col"] = nc.dram_tensor("qb_col", [BPC, 128, J_T], F32, kind="ExternalInput")
    dram["w1c"] = nc.dram_tensor("w1c", [128, K_T], F32R, kind="ExternalInput")
    dram["w2c"] = nc.dram_tensor("w2c", [128, K_T], F32R, kind="ExternalInput")
    dram["w3c"] = nc.dram_tensor("w3c", [128, K_T], F32, kind="ExternalInput")
    dram["ow_t"] = nc.dram_tensor("ow_t", [4 * D, D], F32R, kind="ExternalInput")
    dram["ob_col"] = nc.dram_tensor("ob_col", [128, K_T], F32, kind="ExternalInput")
    dram["out_t"] = nc.dram_tensor("out_t", [BPC, D, LC], F32, kind="ExternalOutput")

    with tile.TileContext(nc) as tc:
        with tc.tile_pool(name="sb", bufs=4) as sb, \
             tc.tile_pool(name="small", bufs=1) as small, \
             tc.tile_pool(name="consts", bufs=1) as cpool, \
             tc.tile_pool(name="psum", bufs=6, space="PSUM") as psum, \
             tc.tile_pool(name="rowps", bufs=2, space="PSUM") as rowps:
            ones_f = small.tile([128, 1], F32, tag="ones_f", bufs=1)
            nc.vector.memset(ones_f[:], 1.0)
            ones_r = cpool.tile([128, 1], F32R)
            nc.vector.tensor_copy(ones_r[:], ones_f[:])
            onesrow_f = small.tile([1, 512], F32, tag="onesrow_f", bufs=1)
            nc.vector.memset(onesrow_f[:], 1.0)
            ones_row = cpool.tile([1, 512], F32R)
            nc.vector.tensor_copy(ones_row[:], onesrow_f[:])
            w1c = cpool.tile([128, K_T], F32R)
            nc.scalar.dma_start(out=w1c[:], in_=dram["w1c"].ap())
            w2c = cpool.tile([128, K_T], F32R)
            nc.scalar.dma_start(out=w2c[:], in_=dram["w2c"].ap())
            w3c = cpool.tile([128, K_T], F32)
            nc.scalar.dma_start(out=w3c[:], in_=dram["w3c"].ap())
            obc = cpool.tile([128, K_T], F32)
            nc.scalar.dma_start(out=obc[:], in_=dram["ob_col"].ap())
            ow = []
            for f in range(F_T):
                t = cpool.tile([128, D], F32R, tag="ow", bufs=F_T, name=f"ow{f}")
                nc.gpsimd.dma_start(out=t[:],
                                    in_=dram["ow_t"].ap()[f * 128:(f + 1) * 128, :])
                ow.append(t)
            consts = (ones_r, ones_row, w1c, w2c, w3c, ow, obc)
            pools = (sb, small, psum, rowps)
            for b in range(BPC):
                fr = _emit_front(nc, tc, pools, consts, dram, b)
                _emit_back(nc, tc, pools, consts, dram, b, fr)

    split_multi_waits(nc)
    return nc


_NC = None


def _get_nc():
    global _NC
    if _NC is None:
        _NC = build()
    return _NC


def make_in_maps(C, Q, cmask, qmask, w, out_w, out_b):
    C = np.asarray(C, dtype=np.float32)
    Q = np.asarray(Q, dtype=np.float32)
    cmask = np.asarray(cmask, dtype=np.float32)
    qmask = np.asarray(qmask, dtype=np.float32)
    w = np.asarray(w, dtype=np.float32)
    out_w = np.asarray(out_w, dtype=np.float32)
    out_b = np.asarray(out_b, dtype=np.float32)

    w1c = np.ascontiguousarray(w[:D].reshape(K_T, 128).T)
    w2c = np.ascontiguousarray(w[D:2 * D].reshape(K_T, 128).T)
    w3c = np.ascontiguousarray(w[2 * D:].reshape(K_T, 128).T)
    ow_t = np.ascontiguousarray(out_w.T)
    ob_col = np.ascontiguousarray(out_b.reshape(K_T, 128).T)

    in_maps = []
    for c in range(NCORES):
        sl = slice(c * BPC, (c + 1) * BPC)
        cb = (cmask[sl] - 1.0) * MASK_BIAS  # [BPC, LC]
        qb = (qmask[sl] - 1.0) * MASK_BIAS  # [BPC, LQ]
        in_maps.append({
            "c_nat": np.ascontiguousarray(C[sl]),
            "c_t": np.ascontiguousarray(C[sl].transpose(0, 2, 1)),
            "q_nat": np.ascontiguousarray(Q[sl]),
            "q_t": np.ascontiguousarray(Q[sl].transpose(0, 2, 1)),
            "cb_col": np.ascontiguousarray(
                cb.reshape(BPC, I_T, 128).transpose(0, 2, 1)),
            "qb_col": np.ascontiguousarray(
                qb.reshape(BPC, J_T, 128).transpose(0, 2, 1)),
            "w1c": w1c, "w2c": w2c, "w3c": w3c,
            "ow_t": ow_t, "ob_col": ob_col,
        })
    return in_maps


def kernel(C, Q, cmask, qmask, w, out_w, out_b):
    nc = _get_nc()
    in_maps = make_in_maps(C, Q, cmask, qmask, w, out_w, out_b)
    res = run_bass_kernel_spmd(nc, in_maps, list(range(NCORES)))
    outs = [res.results[i]["out_t"].transpose(0, 2, 1) for i in range(NCORES)]
    return np.ascontiguousarray(np.concatenate(outs, axis=0))
